# revision 2
# baseline (speedup 1.0000x reference)
"""Two-launch Trainium2 kernel for nn_Linear_act_sp (2:4 activation-sparse linear).

Math identity: out = (x * mask) @ W.T, where mask keeps the top-2 of each
contiguous group of 4 by |x|/s, s = sqrt(max|x| / clip(max|W|, EPS)).

Launch A (stats, data-parallel): per-core abs-max partials of x rows and
W rows -> [1, 8192] per core.  ACT abs + DVE max chains (x and W
interleaved so the W DMA hides under x compute), gpsimd
partition_all_reduce for the 128->1 row fold.  No collective: a
cross-core AllReduce is a barrier that converts core-start stagger into
measured time on every core; the host max-combines the 8 partials and
computes r = 1/s in f32 (bit-matching the reference arithmetic).

Launch B (mask + matmul): host-provided r broadcast [128, 4096].  Per
half-tile [128,2048]: ACT abs -> GpSimd v=|x|*r -> DVE stride-2 pair
max/min tree -> threshold -> is_ge with broadcast-threshold (in place) ->
GpSimd apply x*m (in place).  PE transposes masked x into xmT bf16;
matmuls: stationary = xmT slice bf16, moving = W.T bf16 [128, 512].
The PE serializes the 128-row stationary load with the 512-row moving
stream (no double buffering), so the floor is 640 cyc/matmul; bf16
keeps the LDW at its floor and halves W DMA vs f32.  A visit schedule
(single-o stripes early, o-pairs late, <=4 n-tiles per visit for the 8
psum banks) lets masking of later tile-groups pipeline under earlier
groups' matmuls.
"""

import numpy as np
import ml_dtypes

import concourse.bacc as bacc
import concourse.tile as tile
from concourse import mybir, bass_isa
from concourse.bass_utils import run_bass_kernel_spmd

AluOpType = mybir.AluOpType
ACTF = mybir.ActivationFunctionType
F32 = mybir.dt.float32
BF16 = mybir.dt.bfloat16

N_CORES = 8
N_ROWS = 8192
D_IN = 4096
D_OUT = 4096
ROWS_PER_CORE = N_ROWS // N_CORES      # 1024
WROWS_PER_CORE = D_OUT // N_CORES      # 512
P = 128
NT = ROWS_PER_CORE // P                # 8 row tiles
KT = D_IN // P                         # 32 contraction tiles
H = 2048                               # half width
EPS = np.float32(1e-8)

GROUPS = [(0, 1), (2, 3), (4, 5), (6, 7)]   # n-tiles per rolling group
# Visit schedule: ("s", o, groups) = single-o stripe (halves the W-DMA
# rate for the early small visits), ("p", pair, groups) = o-pair stripe.
# Each visit holds <=4 n-tiles so psum accumulators fit the 8 banks.
VISITS = ([("s", 0, [0]), ("s", 1, [0])]
          + [("s", o, [0, 1]) for o in range(2, 8)]
          + [("s", 0, [1]), ("s", 1, [1])]
          + [("p", pb, [2, 3]) for pb in range(4)])
MASK_AT = {1: 0, 2: 2, 3: 4}   # emit group g's masks before visit index i
TP_AT = {1: 2, 2: 10, 3: 10}   # emit group g's transposes before visit i

_cache = {}
last_results = []


def _build_stats():
    nc = bacc.Bacc("TRN2", target_bir_lowering=False, debug=False,
                   num_devices=N_CORES)
    xs = nc.dram_tensor("xs", [ROWS_PER_CORE, D_IN], F32, kind="ExternalInput")
    ws = nc.dram_tensor("ws", [WROWS_PER_CORE, D_IN], F32,
                        kind="ExternalInput")
    mx = nc.dram_tensor("mx", [1, 2 * D_IN], F32, kind="ExternalOutput")

    with tile.TileContext(nc) as tc:
        with tc.tile_pool(name="sx", bufs=3) as sxpool, \
             tc.tile_pool(name="sw", bufs=2) as swpool, \
             tc.tile_pool(name="sacc", bufs=1) as sapool:
            acc = sapool.tile([P, 2 * D_IN], F32, tag="acc")
            # interleave the x and W chains so the W DMA overlaps x compute
            xt = []
            wt0 = None
            for t in range(NT):
                ti = sxpool.tile([P, D_IN], F32, tag="sxt", name=f"sxt{t}")
                nc.sync.dma_start(ti[:], xs.ap()[t * P:(t + 1) * P, :])
                nc.scalar.activation(ti[:], ti[:], ACTF.Abs)
                if t == 1:
                    nc.vector.tensor_tensor(acc[:, 0:D_IN], xt[0][:], ti[:],
                                            op=AluOpType.max)
                elif t > 1:
                    nc.vector.tensor_tensor(acc[:, 0:D_IN], acc[:, 0:D_IN],
                                            ti[:], op=AluOpType.max)
                xt.append(ti)
                if t % 2 == 0 and t // 2 < 4:
                    w = t // 2
                    wi = swpool.tile([P, D_IN], F32, tag="swt",
                                     name=f"swt{w}")
                    nc.sync.dma_start(wi[:], ws.ap()[w * P:(w + 1) * P, :])
                    nc.scalar.activation(wi[:], wi[:], ACTF.Abs)
                    if w == 1:
                        nc.vector.tensor_tensor(acc[:, D_IN:], wt0[:],
                                                wi[:], op=AluOpType.max)
                    elif w > 1:
                        nc.vector.tensor_tensor(acc[:, D_IN:],
                                                acc[:, D_IN:], wi[:],
                                                op=AluOpType.max)
                    else:
                        wt0 = wi
            nc.gpsimd.partition_all_reduce(
                acc[:, D_IN:], acc[:, D_IN:], channels=P,
                reduce_op=bass_isa.ReduceOp.max)
            nc.gpsimd.partition_all_reduce(
                acc[:, 0:D_IN], acc[:, 0:D_IN], channels=P,
                reduce_op=bass_isa.ReduceOp.max)
            nc.sync.dma_start(mx.ap()[:, :], acc[0:1, :])
    nc.compile()
    return nc


def _build_main():
    nc = bacc.Bacc("TRN2", target_bir_lowering=False, debug=False,
                   num_devices=N_CORES)
    xs = nc.dram_tensor("xs", [ROWS_PER_CORE, D_IN], F32, kind="ExternalInput")
    wt = nc.dram_tensor("wt", [D_IN, D_OUT], BF16, kind="ExternalInput")
    rr = nc.dram_tensor("rr", [P, D_IN], F32, kind="ExternalInput")
    ident = nc.dram_tensor("ident", [P, P], F32, kind="ExternalInput")
    ys = nc.dram_tensor("ys", [ROWS_PER_CORE, D_OUT], F32,
                        kind="ExternalOutput")

    with tile.TileContext(nc) as tc:
        with tc.tile_pool(name="misc", bufs=1) as mpool, \
             tc.tile_pool(name="xh", bufs=6) as xhpool, \
             tc.tile_pool(name="mva", bufs=2) as vpool, \
             tc.tile_pool(name="mab", bufs=2) as abpool, \
             tc.tile_pool(name="mpq", bufs=2) as pqpool, \
             tc.tile_pool(name="mt", bufs=4) as tpool, \
             tc.tile_pool(name="xmT", bufs=1) as xTpool, \
             tc.tile_pool(name="wst", bufs=10) as wpool, \
             tc.tile_pool(name="ot", bufs=6) as opool, \
             tc.tile_pool(name="ps", bufs=8, space="PSUM") as pspool:
            id_t = mpool.tile([P, P], F32, tag="ident")
            nc.sync.dma_start(id_t[:], ident.ap()[:, :])
            rho_rep = mpool.tile([P, D_IN], F32, tag="rhorep")
            nc.sync.dma_start(rho_rep[:], rr.ap()[:, :])

            xmT = xTpool.tile([P, NT * D_IN], BF16, tag="xmT")

            def mask_half(n, h):
                """Mask n-tile n half h; ACT abs, GpSimd mults, DVE
                compares.  Returns the x tile, masked in place (f32)."""
                c0 = h * H
                xh = xhpool.tile([P, H], F32, tag="xh", name=f"xh{n}_{h}")
                nc.sync.dma_start(
                    xh[:], xs.ap()[n * P:(n + 1) * P, c0:c0 + H])
                v = vpool.tile([P, H], F32, tag="v", name=f"v{n}_{h}")
                ab = abpool.tile([P, H], F32, tag="ab", name=f"a{n}_{h}")
                nc.scalar.activation(ab[:], xh[:], ACTF.Abs)
                nc.gpsimd.tensor_tensor(v[:], ab[:], rho_rep[:, c0:c0 + H],
                                        op=AluOpType.mult)
                # stride-2 pair max/min: pp[2g]=max(a,b), pp[2g+1]=max(c,d)
                v2 = v[:].rearrange("p (g m) -> p g m", m=2)
                pp = pqpool.tile([P, H // 2], F32, tag="pq",
                                 name=f"p{n}_{h}")
                nc.vector.tensor_tensor(pp[:], v2[:, :, 0], v2[:, :, 1],
                                        op=AluOpType.max)
                p2 = pp[:].rearrange("p (g m) -> p g m", m=2)
                t1 = tpool.tile([P, H // 4], F32, tag="t1",
                                name=f"t1_{n}{h}")
                nc.vector.tensor_tensor(t1[:], p2[:, :, 0], p2[:, :, 1],
                                        op=AluOpType.min)
                qq = pqpool.tile([P, H // 2], F32, tag="pq",
                                 name=f"q{n}_{h}")
                nc.vector.tensor_tensor(qq[:], v2[:, :, 0], v2[:, :, 1],
                                        op=AluOpType.min)
                q2 = qq[:].rearrange("p (g m) -> p g m", m=2)
                t2 = tpool.tile([P, H // 4], F32, tag="t2",
                                name=f"t2_{n}{h}")
                nc.vector.tensor_tensor(t2[:], q2[:, :, 0], q2[:, :, 1],
                                        op=AluOpType.max)
                nc.vector.tensor_tensor(t1[:], t1[:], t2[:],
                                        op=AluOpType.max)
                # m = (v >= thr) in place into v (broadcast thr), then
                # xm = x*m in place into xh
                v4 = v[:].rearrange("p (g m) -> p g m", m=4)
                nc.vector.tensor_tensor(
                    v4, v4, t1[:].to_broadcast((P, H // 4, 4)),
                    op=AluOpType.is_ge)
                nc.gpsimd.tensor_tensor(xh[:], xh[:], v[:],
                                        op=AluOpType.mult)
                return xh

            def transpose_half(n, h, xm):
                """PE transpose xm f32 -> psum -> ACT drain to xmT bf16."""
                c0 = h * H
                for kb in range(H // 512):
                    ps = pspool.tile([P, 512], F32, tag="ps",
                                     name=f"tp{n}_{h}_{kb}")
                    for j in range(4):
                        nc.tensor.transpose(
                            ps[:, j * P:(j + 1) * P],
                            xm[:, kb * 512 + j * P:kb * 512 + (j + 1) * P],
                            id_t[:])
                    dst0 = n * D_IN + c0 + kb * 512
                    nc.scalar.activation(xmT[:, dst0:dst0 + 512], ps[:],
                                         ACTF.Copy)

            def stripe_one(o, tiles, pfx):
                """Single-o stripe (one psum bank per tile)."""
                psn = {n: pspool.tile([P, 512], F32, tag="ps",
                                      name=f"ps{pfx}_{o}_{n}")
                       for n in tiles}
                for k in range(KT):
                    w_t = wpool.tile([P, 512], BF16, tag="wt",
                                     name=f"w{pfx}_{o}_{k}")
                    nc.sync.dma_start(
                        w_t[:],
                        wt.ap()[k * P:(k + 1) * P, o * 512:(o + 1) * 512])
                    for n in tiles:
                        nc.tensor.matmul(
                            psn[n][:],
                            xmT[:, n * D_IN + k * P:n * D_IN + (k + 1) * P],
                            w_t[:],
                            start=(k == 0), stop=(k == KT - 1))
                for n in tiles:
                    ot = opool.tile([P, 512], F32, tag="ot",
                                    name=f"ot{pfx}_{o}_{n}")
                    nc.scalar.activation(ot[:], psn[n][:], ACTF.Copy)
                    nc.sync.dma_start(
                        ys.ap()[n * P:(n + 1) * P, o * 512:(o + 1) * 512],
                        ot[:])

            def stripe_pair(pb, tiles, pfx):
                """o-pair stripe (two psum banks per tile)."""
                o0 = 2 * pb
                psn = {(n, j): pspool.tile([P, 512], F32, tag="ps",
                                           name=f"ps{pfx}_{pb}_{n}_{j}")
                       for n in tiles for j in range(2)}
                for k in range(KT):
                    wts = []
                    for j in range(2):
                        w_t = wpool.tile([P, 512], BF16, tag="wt",
                                         name=f"w{pfx}_{pb}_{k}_{j}")
                        nc.sync.dma_start(
                            w_t[:],
                            wt.ap()[k * P:(k + 1) * P,
                                    (o0 + j) * 512:(o0 + j + 1) * 512])
                        wts.append(w_t)
                    for n in tiles:
                        lhs = xmT[:, n * D_IN + k * P:n * D_IN + (k + 1) * P]
                        for j in range(2):
                            nc.tensor.matmul(
                                psn[(n, j)][:], lhs, wts[j][:],
                                start=(k == 0), stop=(k == KT - 1))
                for n in tiles:
                    for j in range(2):
                        ot = opool.tile([P, 512], F32, tag="ot",
                                        name=f"ot{pfx}_{pb}_{n}_{j}")
                        nc.scalar.activation(ot[:], psn[(n, j)][:],
                                             ACTF.Copy)
                        nc.sync.dma_start(
                            ys.ap()[n * P:(n + 1) * P,
                                    (o0 + j) * 512:(o0 + j + 1) * 512],
                            ot[:])

            xms = {}

            def mask_group(g):
                for n in GROUPS[g]:
                    xms[n] = [mask_half(n, h) for h in range(2)]

            def transpose_group(g):
                for n in GROUPS[g]:
                    for h in range(2):
                        transpose_half(n, h, xms[n][h])

            mask_group(0)
            transpose_group(0)
            for i, (kind, ob, gs) in enumerate(VISITS):
                for g, at in MASK_AT.items():
                    if at == i:
                        mask_group(g)
                for g, at in TP_AT.items():
                    if at == i:
                        transpose_group(g)
                tiles = [n for g in gs for n in GROUPS[g]]
                if kind == "s":
                    stripe_one(ob, tiles, f"v{i}")
                else:
                    stripe_pair(ob, tiles, f"v{i}")
    nc.compile()
    return nc


def _get(name):
    if name not in _cache:
        _cache[name] = (_build_stats() if name == "stats" else _build_main())
    return _cache[name]


def kernel(x: np.ndarray, W: np.ndarray) -> np.ndarray:
    global last_results
    last_results = []
    bs, seq, d_in = x.shape
    xf = np.ascontiguousarray(x.reshape(-1, d_in), dtype=np.float32)
    W = np.asarray(W, dtype=np.float32)

    x_shards = [np.ascontiguousarray(
        xf[c * ROWS_PER_CORE:(c + 1) * ROWS_PER_CORE]) for c in range(N_CORES)]
    w_shards = [np.ascontiguousarray(
        W[c * WROWS_PER_CORE:(c + 1) * WROWS_PER_CORE])
        for c in range(N_CORES)]
    wt = np.ascontiguousarray(W.T).astype(ml_dtypes.bfloat16)
    ident = np.eye(P, dtype=np.float32)

    # ---- Launch A: per-core abs-max partials ----
    nc_a = _get("stats")
    in_a = [{"xs": x_shards[c], "ws": w_shards[c]} for c in range(N_CORES)]
    res_a = run_bass_kernel_spmd(nc_a, in_a, list(range(N_CORES)))
    last_results.append(res_a)
    mx = np.stack([res_a.results[c]["mx"][0] for c in range(N_CORES)])
    gl = mx.max(axis=0)
    max_act = gl[:D_IN].astype(np.float32)
    max_w = gl[D_IN:].astype(np.float32)

    # exact f32 host glue (same ops/rounding class as the reference)
    s = np.sqrt((max_act / np.clip(max_w, EPS, None)).astype(np.float32)
                ).astype(np.float32)
    r = (np.float32(1.0) / s).astype(np.float32)
    r_rep = np.ascontiguousarray(np.broadcast_to(r, (P, D_IN)),
                                 dtype=np.float32)

    # ---- Launch B: mask + matmul ----
    nc_b = _get("main")
    in_b = [{"xs": x_shards[c], "wt": wt, "rr": r_rep, "ident": ident}
            for c in range(N_CORES)]
    res_b = run_bass_kernel_spmd(nc_b, in_b, list(range(N_CORES)))
    last_results.append(res_b)

    out = np.concatenate([res_b.results[c]["ys"] for c in range(N_CORES)],
                         axis=0)
    return out.reshape(bs, seq, D_OUT)
